# revision 31
# baseline (speedup 1.0000x reference)
"""Multi-Head Latent Attention (DeepSeek-style MLA) on 8 TRN2 NeuronCores.

Sharding: core c handles batch b = c//2 and query rows [ (c%2)*S/2, (c%2+1)*S/2 ).
Each core computes the full KV-side projections for its batch (duplicated between
the two cores sharing a batch) and the Q-side projections / attention / output
projection for its query half. No collectives; the host gathers the 8 output
shards.

Layout strategy: activations are kept feature-major ("transposed", [feature, seq])
so every matmul's contraction dim lands on SBUF partitions. Attention output is
produced directly as attT[h*128+d, q] (v as stationary operand, expT as moving),
which is exactly the lhsT layout the output projection needs - no PE transposes
anywhere. Softmax skips the max-subtraction (scores here are O(1); exp is safe)
and the denominator comes from an all-ones stationary matmul over expT.

RoPE is folded into companion weight matrices host-side:
  rope(x)[2i]   = x[2i] cos_i - x[2i+1] sin_i
  rope(x)[2i+1] = x[2i+1] cos_i + x[2i] sin_i
so with xr = x @ Wr where Wr[:,2i] = -W[:,2i+1], Wr[:,2i+1] = W[:,2i]:
  rope(x @ W) = (x @ W) * cosP + (x @ Wr) * sinP   (pure elementwise).

All matmuls run in bf16 (fp32 PSUM accumulation): same 1 cycle/row streaming
rate as float32r at N>=256, but LDWEIGHTS gets fast-weight-load (disabled for
fp32 dtypes) so the per-matmul weight swap hides under the previous matmul,
and DMA bytes / SBUF footprint halve. qT and the zero-padded per-head qrT2
stay resident in SBUF; q_rot uses a full 128-row krT stationary against
qrT2's zero pad rows (64-row stationaries measured +110ns/matmul).

Scheduling notes (each measured on HW):
- A DMA occupies its issuing engine's queue for the whole transfer, so
  traffic is spread: P1 weights + tables on the ACT ring, activation
  streams / attT / outputs on the SP ring, attention-phase weight
  prefetches and SBUF shuffles on the gpsimd SWDGE ring.
- kT/v for head h+1 are produced during head h so their PSUM evictions
  (ACT engine) sit ahead of the slow [128,512] reciprocals in engine FIFOs.
- psS has 3 banks so the scheduler can hoist the next kc's score matmul
  over the ~800ns exp latency; phase-1/3 chains share its tag, the
  produce_v/k chains take the single psA bank, att@v and the softmax
  denominator accumulate in psG/psZ (2+2).
"""

import sys
import numpy as np

sys.path.insert(0, "/opt/trn_rl_repo")

from contextlib import ExitStack  # noqa: E402

import concourse.bass as bass  # noqa: E402
import concourse.mybir as mybir  # noqa: E402
import concourse.tile as tile  # noqa: E402

F32 = mybir.dt.float32
BF = mybir.dt.bfloat16
AF = mybir.ActivationFunctionType
ALU = mybir.AluOpType

# Max sync-waits walrus CoreV3 codegen accepts on one instruction. The stock
# TileContext tail-drain attaches one wait per outstanding semaphore to a
# single Drain, which this walrus build rejects ("Too many sync wait
# commands"); split across several drains instead.
_MAX_WAITS_PER_INST = 1


def _split_excess_waits_json(bir_json):
    """Walrus CoreV3 codegen rejects instructions carrying more than one
    sync-wait. Tile freely attaches several. Rewrite the BIR: keep one wait on
    the instruction, move the rest onto NoOps inserted just before it on the
    same engine (a same-engine wait that fires earlier is strictly safe).
    Updates are left untouched - they must fire at instruction completion."""
    import orjson

    bir = orjson.loads(bir_json)
    n = 0
    for fn in bir.get("functions", []):
        for bb in fn.get("blocks", []):
            out = []
            for inst in bb.get("instructions", []):
                si = inst.get("sync_info")
                waits = (si or {}).get("on_wait") or []
                if len(waits) > _MAX_WAITS_PER_INST:
                    keep = waits[-_MAX_WAITS_PER_INST:]
                    for w in waits[:-_MAX_WAITS_PER_INST]:
                        out.append({
                            "name": f"I-WS{n}",
                            "opcode": "NoOp",
                            "engine": inst["engine"],
                            "ins": [],
                            "outs": [],
                            "sync_info": {"on_update": [], "on_wait": [w]},
                        })
                        n += 1
                    si["on_wait"] = keep
                out.append(inst)
            bb["instructions"] = out
    return orjson.dumps(bir)


_COMPILE_HOOKED = False


def _install_wait_split_hook():
    """Wrap compile_bir_kernel (both the bass_utils global and the name
    bass2jax imported) so every BIR headed to walrus gets the wait split."""
    global _COMPILE_HOOKED
    if _COMPILE_HOOKED:
        return
    from concourse import bass2jax, bass_utils

    orig = bass_utils.compile_bir_kernel

    def hooked(bir_json, tmpdir, neff_name="file.neff"):
        return orig(_split_excess_waits_json(bir_json), tmpdir, neff_name=neff_name)

    bass_utils.compile_bir_kernel = hooked
    bass2jax.compile_bir_kernel = hooked
    _COMPILE_HOOKED = True


class SplitDrainTileContext(tile.TileContext):
    def _drain_and_barrier(self, tick_clock, wait_clock):
        from concourse.tile_scheduler import N_PROCS
        from concourse.vector_clock import ScopedClock, VectorClock

        g = tick_clock.global_clock
        vals = [g[p] for p in range(N_PROCS)]
        nz = [p for p in range(N_PROCS) if vals[p] > 0]
        groups = [nz[i:i + _MAX_WAITS_PER_INST]
                  for i in range(0, len(nz), _MAX_WAITS_PER_INST)] or [[]]
        for grp in groups:
            sub = VectorClock([vals[p] if p in grp else 0 for p in range(N_PROCS)])
            drain_inst = self.nc.sync.drain()
            wait_clock.add_sem_waits(drain_inst.ins, ScopedClock({None: sub}))

        self.nc.all_engine_barrier()
        assert self.sems is not None
        popped = self.nc._tile_sem_poison_stack.pop()
        assert popped is self._sem_poison
        self.nc.clear_and_free_semaphores(list(self.sems.allocated().values()))
        self.nc.all_engine_barrier()


# ----------------------------------------------------------------------------
# Config
# ----------------------------------------------------------------------------

class Cfg:
    def __init__(self, E=2048, DM=2048, H=16, DC=512, DC1=1536, S=2048, Q=1024,
                 QT=512, bf16=True):
        self.E, self.DM, self.H, self.DC, self.DC1 = E, DM, H, DC, DC1
        self.S, self.Q, self.QT = S, Q, QT
        self.DR = 64          # rotary dim (fixed by the problem)
        self.DH = 128         # nope head dim (fixed: DM // H)
        self.bf16 = bf16
        assert DM == H * self.DH and H % 2 == 0
        assert E % 128 == 0 and DC % 128 == 0 and DC1 % 128 == 0
        assert S % 128 == 0
        assert Q % QT == 0 and Q % 128 == 0 and QT <= 512
        self.EC = E // 128        # embed chunks
        self.CC = DC // 128       # c_kv chunks
        self.C1C = DC1 // 128     # c_q chunks
        self.KC = S // 128        # key chunks (128-wide)
        self.ST = min(512, S)     # seq tile for phase 1
        self.STN = S // self.ST
        self.NT = min(512, S)     # kT free tile
        self.NTN = S // self.NT
        self.QTN = Q // QT
        self.MT = min(512, DM)    # out-proj free tile
        self.MTN = DM // self.MT
        self.QON = Q // 128       # out-proj q tiles


FULL = Cfg()


# ----------------------------------------------------------------------------
# Program builder (single-core SPMD program)
# ----------------------------------------------------------------------------

def build_program(cfg: Cfg, has_buv=True, has_bo=True):
    c = cfg
    FR = BF if getattr(cfg, "bf16", False) else mybir.dt.float32r
    nc = bass.Bass()
    r = lambda ap: ap  # noqa: E731

    # -- DRAM parameters -----------------------------------------------------
    xt = nc.dram_tensor("xt", [c.E, c.S], FR, kind="ExternalInput")
    xtq = nc.dram_tensor("xtq", [c.E, c.Q], FR, kind="ExternalInput")
    cosq = nc.dram_tensor("cosq", [128, c.Q], F32, kind="ExternalInput")
    sinq = nc.dram_tensor("sinq", [128, c.Q], F32, kind="ExternalInput")
    # rows 0:64 cos table, rows 64:128 sin table (packed for the fused k-rope)
    cossink = nc.dram_tensor("cossink", [128, c.S], F32, kind="ExternalInput")
    wdq = nc.dram_tensor("wdq", [c.E, c.DC1], FR, kind="ExternalInput")
    bdq = nc.dram_tensor("bdq", [c.DC1], F32, kind="ExternalInput")
    wdkv = nc.dram_tensor("wdkv", [c.E, c.DC], FR, kind="ExternalInput")
    bdkv = nc.dram_tensor("bdkv", [c.DC], F32, kind="ExternalInput")
    wuq = nc.dram_tensor("wuq", [c.DC1, c.DM], FR, kind="ExternalInput")
    buq = nc.dram_tensor("buq", [c.DM], F32, kind="ExternalInput")
    wrq = nc.dram_tensor("wrq", [c.DC1, c.H * c.DR], FR, kind="ExternalInput")
    brq = nc.dram_tensor("brq", [c.H * c.DR], F32, kind="ExternalInput")
    wrqr = nc.dram_tensor("wrqr", [c.DC1, c.H * c.DR], FR, kind="ExternalInput")
    brqr = nc.dram_tensor("brqr", [c.H * c.DR], F32, kind="ExternalInput")
    wrk = nc.dram_tensor("wrk", [c.E, 2 * c.DR], FR, kind="ExternalInput")
    brk = nc.dram_tensor("brk", [2 * c.DR], F32, kind="ExternalInput")
    wuk = nc.dram_tensor("wuk", [c.DC, c.DM], FR, kind="ExternalInput")
    buk = nc.dram_tensor("buk", [c.DM], F32, kind="ExternalInput")
    wuv = nc.dram_tensor("wuv", [c.DC, c.DM], FR, kind="ExternalInput")
    buv = nc.dram_tensor("buv", [c.DM], FR, kind="ExternalInput")
    wo = nc.dram_tensor("wo", [c.DM, c.DM], FR, kind="ExternalInput")
    bo = nc.dram_tensor("bo", [c.DM], FR, kind="ExternalInput")
    ones_d = nc.dram_tensor("ones_in", [128, 128], FR, kind="ExternalInput")
    out = nc.dram_tensor("out", [c.Q, c.DM], F32, kind="ExternalOutput")
    attd = nc.dram_tensor("attT_scratch", [c.DM, c.Q], BF)

    with SplitDrainTileContext(nc) as tc, ExitStack() as ctx:
        # weights / tables / small SBUF-SBUF shuffles ride the ACT HWDGE
        # ring; activation streams and output writes ride the SP ring; the
        # bulk qrT2 shuffle rides the gpsimd SWDGE ring. Rings drain in
        # parallel, so bulk weight loads never head-block the activation
        # stream (and vice versa).
        wdma = nc.scalar.dma_start
        adma = nc.sync.dma_start
        gdma = nc.gpsimd.dma_start

        # -- persistent pools ------------------------------------------------
        consts = ctx.enter_context(tc.tile_pool(name="consts", bufs=1))
        res = ctx.enter_context(tc.tile_pool(name="res", bufs=1))

        ckvT = res.tile([128, c.CC, c.S], FR, tag="ckvT")     # c_kv^T
        krT = res.tile([128, c.S], BF, tag="krT")             # roped k_rot^T, dup rows
        qT = res.tile([128, c.H, c.Q], FR, tag="qT")          # scaled q^T

        def load_pcol(name, vec, n):
            # [n*128] dram vector -> [128, n] sbuf (per-partition scalars)
            t = consts.tile([128, n], F32, tag=name)
            wdma(out=t, in_=vec.rearrange("(c p) -> p c", p=128))
            return t

        # PSUM pools (8 banks total: 2+2+2+2)
        psA = ctx.enter_context(tc.tile_pool(name="psA", bufs=1, space="PSUM"))
        psS = ctx.enter_context(tc.tile_pool(name="psS", bufs=3, space="PSUM"))
        psG = ctx.enter_context(tc.tile_pool(name="psG", bufs=2, space="PSUM"))
        psZ = ctx.enter_context(tc.tile_pool(name="psZ", bufs=2, space="PSUM"))

        paq = ctx.enter_context(tc.tile_pool(name="paq", bufs=1))
        # per-head zero-padded q_rot: rows 0:64 = head h's roped q_rot, rows
        # 64:128 = 0. Score matmuls can then use the full 128-row krT
        # stationary (the dup rows multiply zeros), which keeps LDWEIGHTS on
        # the fast path - 64-row stationaries measured +110ns per matmul.
        qrT2 = paq.tile([128, c.H, c.Q], BF, tag="qrT2")

        # head 0/1 attention weights live below the phase pools so their
        # DMAs (issued during 1c) never wait on an aliased zone
        hw = ctx.enter_context(tc.tile_pool(name="hw", bufs=3))

        # ==================================================================
        # Phase 1a: c_kv^T and roped k_rot^T over the full sequence
        # ==================================================================
        with tc.tile_pool(name="p1ax", bufs=2 * c.EC + 6) as p1ax, \
             tc.tile_pool(name="p1aw", bufs=c.EC) as p1aw, \
             tc.tile_pool(name="p1am", bufs=1) as p1am, \
             tc.tile_pool(name="p1at", bufs=4) as p1at:

            # ACT-ring issue order = need order: the 16 wdkv tiles gate the
            # first matmul chain, bdkv its PSUM eviction, wrk/cossink the
            # k_rot tail; everything else is needed phases later.
            wdkv_t, wrk_t = [], []
            for e in range(c.EC):
                wt = p1aw.tile([128, c.DC], FR, tag="wdkv", name=f"wdkv{e}")
                wdkv_t.append(wt)
            for e in range(c.EC):
                wdma(out=wdkv_t[e], in_=wdkv[e * 128:(e + 1) * 128, :])
            bdkv_sb = load_pcol("bdkv", bdkv, c.CC)
            for e in range(c.EC):
                rt = p1aw.tile([128, 2 * c.DR], FR, tag="wrk", name=f"wrk{e}")
                wrk_t.append(rt)
            for e in range(c.EC):
                wdma(out=wrk_t[e], in_=wrk[e * 128:(e + 1) * 128, :])
            brk_sb = load_pcol("brk", brk, 1)  # rows 0:64 brk, 64:128 companion
            coss_sb = p1am.tile([128, c.S], F32, tag="coss")
            wdma(out=coss_sb, in_=cossink[:, :])
            bdq_sb = load_pcol("bdq", bdq, c.C1C)
            buq_sb = load_pcol("buq", buq, c.H)
            brq_sb = load_pcol("brq", brq, c.H // 2)
            brqr_sb = load_pcol("brqr", brqr, c.H // 2)
            buk_sb = load_pcol("buk", buk, c.H)
            buv_sb = consts.tile([1, c.DM], FR, tag="buv")
            wdma(out=buv_sb, in_=buv[:].unsqueeze(0))
            bo_sb = consts.tile([1, c.DM], FR, tag="bo")
            wdma(out=bo_sb, in_=bo[:].unsqueeze(0))
            ones128 = consts.tile([128, 128], FR, tag="ones128")
            wdma(out=ones128, in_=ones_d[:, :])
            ones1 = ones128[0:1, :]

            for st in range(c.STN):
                ssl = bass.ts(st, c.ST)
                xts = []
                for e in range(c.EC):
                    t = p1ax.tile([128, c.ST], FR, tag="xt")
                    adma(out=t, in_=xt[e * 128:(e + 1) * 128, ssl])
                    xts.append(t)
                for ct in range(c.CC):
                    ps = psS.tile([128, c.ST], F32, tag="s")
                    for e in range(c.EC):
                        nc.tensor.matmul(ps, r(wdkv_t[e][:, ct * 128:(ct + 1) * 128]),
                                         r(xts[e]), start=(e == 0), stop=(e == c.EC - 1))
                    nc.vector.tensor_scalar_add(ckvT[:, ct, ssl], ps,
                                                bdkv_sb[:, ct:ct + 1])
                # k_rot: one 128-col stationary covers A rows (0:64, cos
                # part) and companion Ar rows (64:128, sin part) in one psum
                ps = psS.tile([128, c.ST], F32, tag="s")
                for e in range(c.EC):
                    nc.tensor.matmul(ps, r(wrk_t[e]), r(xts[e]),
                                     start=(e == 0), stop=(e == c.EC - 1))
                tmp = p1at.tile([128, c.ST], F32, tag="ktmp")
                nc.vector.scalar_tensor_tensor(tmp[0:64, :], ps[0:64, :],
                                               brk_sb[0:64, :],
                                               coss_sb[0:64, ssl], ALU.add, ALU.mult)
                nc.vector.scalar_tensor_tensor(tmp[64:128, :], ps[64:128, :],
                                               brk_sb[64:128, :],
                                               coss_sb[64:128, ssl], ALU.add, ALU.mult)
                tmp2 = p1at.tile([64, c.ST], F32, tag="ktmp2")
                gdma(out=tmp2, in_=tmp[64:128, :])
                nc.vector.tensor_add(krT[0:64, ssl], tmp[0:64, :], tmp2)
            # duplicate kr rows: rows 64:128 are the stationary rows that
            # multiply qrT2's zero rows - any finite value works, a copy is
            # the cheapest way to guarantee finite.
            gdma(out=krT[64:128, :], in_=krT[0:64, :])

        # zero qrT2's pad rows off the k_rot tail's critical path; the rot
        # matmuls (attention) are the only readers
        nc.gpsimd.memset(qrT2[64:128, :, :], 0.0)

        # pre-issue head 0/1 attention weights: the hw zone aliases nothing,
        # so these flow on the ACT ring during 1b/1c
        wuv_p = hw.tile([128, c.CC, 256], FR, tag="wuv", name="wuv_pre")
        wdma(out=wuv_p,
             in_=wuv.rearrange("(cc p) m -> p cc m", p=128)[:, :, 0:256])
        wuk_p0 = hw.tile([128, c.CC, 128], FR, tag="wuk", name="wuk_pre0")
        wdma(out=wuk_p0,
             in_=wuk.rearrange("(cc p) m -> p cc m", p=128)[:, :, 0:128])
        wuk_p1 = hw.tile([128, c.CC, 128], FR, tag="wuk", name="wuk_pre1")
        wdma(out=wuk_p1,
             in_=wuk.rearrange("(cc p) m -> p cc m", p=128)[:, :, 128:256])

        with tc.tile_pool(name="p1bx", bufs=c.QTN * c.EC + 2) as p1bx, \
             tc.tile_pool(name="p1bw", bufs=2) as p1bw:
            # 1b's activations: fresh zone, so these queue dep-free on the
            # SP ring right behind 1a's xt stream
            xqs = {}
            for qt in range(c.QTN):
                qsl = bass.ts(qt, c.QT)
                for e in range(c.EC):
                    t = p1bx.tile([128, c.QT], FR, tag="xq")
                    adma(out=t, in_=xtq[e * 128:(e + 1) * 128, qsl])
                    xqs[qt, e] = t

            # ==============================================================
            # Phase 1b/1c: c_q^T, then q^T (scaled) and roped q_rot^T
            # ==============================================================
            with tc.tile_pool(name="pcq", bufs=1) as pcq, \
                 tc.tile_pool(name="p1cm", bufs=1) as p1cm, \
                 tc.tile_pool(name="p1cw", bufs=2) as p1cw:
                cqT = pcq.tile([128, c.C1C, c.Q], FR, tag="cqT")

                cosq_sb = p1cm.tile([128, c.Q], F32, tag="cosq")
                sinq_sb = p1cm.tile([128, c.Q], F32, tag="sinq")
                wdma(out=cosq_sb, in_=cosq[:, :])
                wdma(out=sinq_sb, in_=sinq[:, :])

                for ct in range(c.C1C):
                    wdq_ct = p1bw.tile([128, c.EC, 128], FR, tag="wdq")
                    wdma(
                        out=wdq_ct,
                        in_=wdq.rearrange("(e p) m -> p e m", p=128)[:, :, ct * 128:(ct + 1) * 128])
                    for qt in range(c.QTN):
                        qsl = bass.ts(qt, c.QT)
                        ps = psS.tile([128, c.QT], F32, tag="s")
                        for e in range(c.EC):
                            nc.tensor.matmul(ps, r(wdq_ct[:, e, :]), r(xqs[qt, e]),
                                             start=(e == 0), stop=(e == c.EC - 1))
                        nc.vector.tensor_scalar_add(cqT[:, ct, qsl], ps,
                                                    bdq_sb[:, ct:ct + 1])

                with tc.tile_pool(name="p1ct", bufs=4) as p1ct:
                    for h in range(c.H):
                        wuq_h = p1cw.tile([128, c.C1C, 128], FR, tag="wuq")
                        wdma(
                            out=wuq_h,
                            in_=wuq.rearrange("(cc p) m -> p cc m", p=128)[:, :, h * 128:(h + 1) * 128])
                        for qt in range(c.QTN):
                            qsl = bass.ts(qt, c.QT)
                            ps = psS.tile([128, c.QT], F32, tag="s")
                            for ct in range(c.C1C):
                                nc.tensor.matmul(ps, r(wuq_h[:, ct, :]), r(cqT[:, ct, qsl]),
                                                 start=(ct == 0), stop=(ct == c.C1C - 1))
                            nc.vector.tensor_scalar_add(qT[:, h, qsl], ps,
                                                        buq_sb[:, h:h + 1])
                    for hp in range(c.H // 2):
                        wrq_hp = p1cw.tile([128, c.C1C, 128], FR, tag="wrq")
                        wdma(
                            out=wrq_hp,
                            in_=wrq.rearrange("(cc p) m -> p cc m", p=128)[:, :, hp * 128:(hp + 1) * 128])
                        wrqr_hp = p1cw.tile([128, c.C1C, 128], FR, tag="wrqr")
                        wdma(
                            out=wrqr_hp,
                            in_=wrqr.rearrange("(cc p) m -> p cc m", p=128)[:, :, hp * 128:(hp + 1) * 128])
                        for qt in range(c.QTN):
                            qsl = bass.ts(qt, c.QT)
                            psa = psS.tile([128, c.QT], F32, tag="s")
                            for ct in range(c.C1C):
                                nc.tensor.matmul(psa, r(wrq_hp[:, ct, :]), r(cqT[:, ct, qsl]),
                                                 start=(ct == 0), stop=(ct == c.C1C - 1))
                            psar = psS.tile([128, c.QT], F32, tag="s")
                            for ct in range(c.C1C):
                                nc.tensor.matmul(psar, r(wrqr_hp[:, ct, :]), r(cqT[:, ct, qsl]),
                                                 start=(ct == 0), stop=(ct == c.C1C - 1))
                            tmp = p1ct.tile([128, c.QT], F32, tag="qtmp")
                            nc.vector.scalar_tensor_tensor(tmp, psa, brq_sb[:, hp:hp + 1],
                                                           cosq_sb[:, qsl], ALU.add, ALU.mult)
                            qrp = p1ct.tile([128, c.QT], BF, tag="qrp")
                            nc.vector.scalar_tensor_tensor(qrp, psar,
                                                           brqr_sb[:, hp:hp + 1],
                                                           sinq_sb[:, qsl], ALU.add, ALU.mult)
                            nc.vector.tensor_add(qrp, qrp, tmp)
                            # pair-packed rows -> per-head zero-padded layout
                            gdma(out=qrT2[0:64, 2 * hp, qsl], in_=qrp[0:64, :])
                            gdma(out=qrT2[0:64, 2 * hp + 1, qsl], in_=qrp[64:128, :])

        # ==================================================================
        # Phase 2: per-head attention, kT/v produced one head ahead so their
        # PSUM evictions sit before the reciprocals in engine FIFOs (v casts
        # go to the scalar engine for the same reason). Phase 3's first four
        # chains run inside head 15 to cover its tail.
        # ==================================================================
        wo_pre = []
        with tc.tile_pool(name="ow", bufs=c.H + 2) as ow, \
             tc.tile_pool(name="oo", bufs=2) as oo, \
             tc.tile_pool(name="oa", bufs=c.QON) as oa:
            aqs = []

            def p3_chain(wo_t, mt, qo):
                msl = bass.ts(mt, c.MT)
                ps = psS.tile([128, c.MT], F32, tag="s")
                for hc in range(c.H):
                    nc.tensor.matmul(ps, r(aqs[qo][:, hc, :]), r(wo_t[hc]),
                                     start=(hc == 0),
                                     stop=(not has_bo and hc == c.H - 1))
                if has_bo:
                    nc.tensor.matmul(ps, r(ones1), r(bo_sb[:, msl]),
                                     start=False, stop=True)
                osb = oo.tile([128, c.MT], F32, tag="osb")
                nc.vector.tensor_copy(osb, ps)
                adma(out=out[qo * 128:(qo + 1) * 128, msl], in_=osb)

            with tc.tile_pool(name="hk", bufs=2) as hk, \
                 tc.tile_pool(name="hv", bufs=4) as hv, \
                 tc.tile_pool(name="he", bufs=4) as he, \
                 tc.tile_pool(name="hr", bufs=2) as hr:

                v_tiles, k_tiles = {}, {}

                def produce_v(hp, pre=None):
                    if pre is None:
                        wuv_hp = hw.tile([128, c.CC, 256], FR, tag="wuv")
                        adma(
                            out=wuv_hp,
                            in_=wuv.rearrange("(cc p) m -> p cc m", p=128)[:, :, hp * 256:(hp + 1) * 256])
                    else:
                        wuv_hp = pre
                    v0 = hv.tile([128, c.KC, 128], FR, tag="vh")
                    v1 = hv.tile([128, c.KC, 128], FR, tag="vh")
                    for st in range(c.KC):
                        psf = psA.tile([128, c.NT], F32, tag="ps")
                        ps = psf[:, 0:256]
                        for cc in range(c.CC):
                            nc.tensor.matmul(ps, r(ckvT[:, cc, st * 128:(st + 1) * 128]),
                                             r(wuv_hp[:, cc, :]),
                                             start=(cc == 0),
                                             stop=(not has_buv and cc == c.CC - 1))
                        if has_buv:
                            nc.tensor.matmul(ps, r(ones1),
                                             r(buv_sb[:, hp * 256:(hp + 1) * 256]),
                                             start=False, stop=True)
                        nc.scalar.copy(v0[:, st, :], ps[:, 0:128])
                        nc.scalar.copy(v1[:, st, :], ps[:, 128:256])
                    v_tiles[2 * hp], v_tiles[2 * hp + 1] = v0, v1

                def produce_k(h, pre=None):
                    if pre is None:
                        wuk_h = hw.tile([128, c.CC, 128], FR, tag="wuk")
                        adma(
                            out=wuk_h,
                            in_=wuk.rearrange("(cc p) m -> p cc m", p=128)[:, :, h * 128:(h + 1) * 128])
                    else:
                        wuk_h = pre
                    kT = hk.tile([128, c.S], FR, tag="kT")
                    for nt in range(c.NTN):
                        nsl = bass.ts(nt, c.NT)
                        ps = psA.tile([128, c.NT], F32, tag="ps")
                        for cc in range(c.CC):
                            nc.tensor.matmul(ps, r(wuk_h[:, cc, :]), r(ckvT[:, cc, nsl]),
                                             start=(cc == 0), stop=(cc == c.CC - 1))
                        nc.scalar.add(kT[:, nsl], ps, buk_sb[:, h:h + 1])
                    k_tiles[h] = kT

                produce_v(0, pre=wuv_p)
                produce_k(0, pre=wuk_p0)
                for h in range(c.H):
                    if h % 2 == 0 and h + 2 < c.H:
                        produce_v(h // 2 + 1)
                    if h + 1 < c.H:
                        produce_k(h + 1, pre=wuk_p1 if h == 0 else None)
                    # one wo prefetch per head for phase 3's mt=0
                    t = ow.tile([128, c.MT], FR, tag="wo")
                    adma(out=t, in_=wo[h * 128:(h + 1) * 128, 0:c.MT])
                    wo_pre.append(t)

                    kT = k_tiles.pop(h)
                    vh = v_tiles.pop(h)
                    for qt in range(c.QTN):
                        qsl = bass.ts(qt, c.QT)
                        gps = psG.tile([128, c.QT], F32, tag="g")
                        zps = psZ.tile([128, c.QT], F32, tag="z")
                        ets, ess = [], []
                        for kc in range(c.KC):
                            ksl = bass.ts(kc, 128)
                            sps = psS.tile([128, c.QT], F32, tag="s")
                            nc.tensor.matmul(sps, r(kT[:, ksl]), r(qT[:, h, qsl]),
                                             start=True, stop=False)
                            nc.tensor.matmul(sps, r(krT[:, ksl]),
                                             r(qrT2[:, h, qsl]),
                                             start=False, stop=True)
                            et = he.tile([128, c.QT], FR, tag="e")
                            nc.scalar.activation(et, sps, AF.Exp)
                            nc.tensor.matmul(gps, r(vh[:, kc, :]), r(et),
                                             start=(kc == 0), stop=(kc == c.KC - 1))
                            ets.append(et)
                            if kc % 2 == 1:
                                # denominator at half the PE cost: DVE pair-sums
                                # feed one ones-matmul per TWO key chunks. Each
                                # matmul is emitted two pairs after its sum, so
                                # the PE FIFO never waits on a fresh pair-sum
                                # and only 3 es buffers are ever live.
                                es = he.tile([128, c.QT], FR, tag="es")
                                nc.gpsimd.tensor_add(es, ets[kc - 1], ets[kc])
                                ess.append(es)
                                if len(ess) >= 3:
                                    i = len(ess) - 3
                                    nc.tensor.matmul(zps, r(ones128), r(ess[i]),
                                                     start=(i == 0), stop=False)
                        for i in (len(ess) - 2, len(ess) - 1):
                            nc.tensor.matmul(zps, r(ones128), r(ess[i]),
                                             start=(i == 0), stop=(i == len(ess) - 1))
                        recip = hr.tile([128, c.QT], F32, tag="recip")
                        nc.vector.reciprocal(recip, zps)
                        asb = hr.tile([128, c.QT], BF, tag="attsb")
                        nc.vector.tensor_mul(asb, gps, recip)
                        adma(out=attd[h * 128:(h + 1) * 128, qsl], in_=asb)
                        if h == c.H - 1 and qt == 0:
                            # every head's qt0 slab is in attd: prefetch
                            # phase 3's first stationaries under qt1's work
                            for qo in range(c.QON // 2):
                                aq = oa.tile([128, c.H, 128], BF, tag="attq")
                                adma(
                                    out=aq,
                                    in_=attd.rearrange("(hc p) q -> p hc q", p=128)[:, :, qo * 128:(qo + 1) * 128])
                                aqs.append(aq)

            # ==============================================================
            # Phase 3: output projection  out[q, m] = attT.T @ wo + bo
            # ==============================================================
            for qo in range(c.QON // 2, c.QON):
                aq = oa.tile([128, c.H, 128], BF, tag="attq")
                adma(
                    out=aq,
                    in_=attd.rearrange("(hc p) q -> p hc q", p=128)[:, :, qo * 128:(qo + 1) * 128])
                aqs.append(aq)
            for mt in range(c.MTN):
                if mt == 0:
                    wo_t = wo_pre
                    qos = range(c.QON)
                else:
                    wo_t = []
                    for hc in range(c.H):
                        t = ow.tile([128, c.MT], FR, tag="wo")
                        gdma(out=t, in_=wo[hc * 128:(hc + 1) * 128, bass.ts(mt, c.MT)])
                        wo_t.append(t)
                    qos = range(c.QON)
                for qo in qos:
                    p3_chain(wo_t, mt, qo)

    return nc


# ----------------------------------------------------------------------------
# Host side: input prep, sharding, gather
# ----------------------------------------------------------------------------

def _rope_tables(seq_len, dim, theta=10000.0):
    inv_freq = 1.0 / (theta ** (np.arange(0, dim, 2, dtype=np.float32) / dim))
    t = np.arange(seq_len, dtype=np.float32)
    ang = t[:, None] * inv_freq[None, :]  # [S, dim/2]
    return np.cos(ang).astype(np.float32), np.sin(ang).astype(np.float32)


def _rot_companion_cols(w):
    """wr[..., 2i] = -w[..., 2i+1]; wr[..., 2i+1] = w[..., 2i]."""
    wr = np.empty_like(w)
    wr[..., 0::2] = -w[..., 1::2]
    wr[..., 1::2] = w[..., 0::2]
    return wr


def host_inputs(cfg, sequence, W_dkv, b_dkv, W_dq, b_dq, W_uq, b_uq, W_uk, b_uk,
                W_uv, b_uv, W_rq, b_rq, W_rk, b_rk, W_o, b_o):
    """Build the per-core input maps for the SPMD program."""
    c = cfg
    f = lambda a: np.ascontiguousarray(np.asarray(a, dtype=np.float32))  # noqa: E731
    sequence = f(sequence)
    B = sequence.shape[0]
    scaler = np.float32(1.0 / np.sqrt(c.DH + c.DR))

    cos, sin = _rope_tables(c.S, c.DR)  # [S, 32]
    # rows 2i and 2i+1 both carry table column i
    cosk = np.repeat(cos.T, 2, axis=0)  # [64, S]
    sink = np.repeat(sin.T, 2, axis=0)

    shared = dict(
        wdq=f(W_dq), bdq=f(b_dq),
        wdkv=f(W_dkv), bdkv=f(b_dkv),
        wuq=f(W_uq) * scaler, buq=f(b_uq) * scaler,
        wrq=f(W_rq) * scaler, brq=f(b_rq) * scaler,
        wrqr=_rot_companion_cols(f(W_rq) * scaler),
        brqr=_rot_companion_cols(f(b_rq) * scaler),
        wrk=np.concatenate([f(W_rk), _rot_companion_cols(f(W_rk))], axis=1),
        brk=np.concatenate([f(b_rk), _rot_companion_cols(f(b_rk))], axis=0),
        wuk=f(W_uk), buk=f(b_uk),
        wuv=f(W_uv), buv=f(b_uv),
        wo=f(W_o), bo=f(b_o),
        cossink=np.concatenate([f(cosk), f(sink)], axis=0),
        ones_in=np.ones((128, 128), np.float32),
    )
    shared = {k: np.ascontiguousarray(v) for k, v in shared.items()}
    mm_keys = {"wdq", "wdkv", "wuq", "wrq", "wrqr", "wrk", "wuk", "wuv", "wo",
               "buv", "bo", "ones_in"}
    if getattr(c, "bf16", False):
        import ml_dtypes
        for k in mm_keys:
            shared[k] = shared[k].astype(ml_dtypes.bfloat16)

    n_cores = 2 * B
    in_maps = []
    for core in range(n_cores):
        b, half = core // 2, core % 2
        xtc = np.ascontiguousarray(sequence[b].T)         # [E, S]
        q0 = half * c.Q
        xtqc = np.ascontiguousarray(xtc[:, q0:q0 + c.Q])  # [E, Q]
        cq = np.tile(np.repeat(cos[q0:q0 + c.Q].T, 2, axis=0), (2, 1))  # [128, Q]
        sq = np.tile(np.repeat(sin[q0:q0 + c.Q].T, 2, axis=0), (2, 1))
        m = dict(shared)
        if getattr(c, "bf16", False):
            import ml_dtypes
            xtc = xtc.astype(ml_dtypes.bfloat16)
            xtqc = xtqc.astype(ml_dtypes.bfloat16)
        m.update(xt=xtc, xtq=xtqc,
                 cosq=np.ascontiguousarray(cq), sinq=np.ascontiguousarray(sq))
        in_maps.append(m)
    return in_maps


_PROG_CACHE = {}


def kernel(**inputs) -> np.ndarray:
    from concourse.bass_utils import run_bass_kernel_spmd

    _install_wait_split_hook()

    cfg = FULL
    has_buv = bool(np.any(np.asarray(inputs["b_uv"])))
    has_bo = bool(np.any(np.asarray(inputs["b_o"])))
    key = ("full", has_buv, has_bo)
    if key not in _PROG_CACHE:
        _PROG_CACHE[key] = build_program(cfg, has_buv=has_buv, has_bo=has_bo)
    nc = _PROG_CACHE[key]

    in_maps = host_inputs(cfg, **inputs)
    n = len(in_maps)
    res = run_bass_kernel_spmd(nc, in_maps, list(range(n)))

    B = n // 2
    S = 2 * cfg.Q
    full = np.empty((B, S, cfg.DM), dtype=np.float32)
    for core in range(n):
        b, half = core // 2, core % 2
        full[b, half * cfg.Q:(half + 1) * cfg.Q, :] = res.results[core]["out"]
    return full


# revision 32
# speedup vs baseline: 1.0149x; 1.0149x over previous
"""Multi-Head Latent Attention (DeepSeek-style MLA) on 8 TRN2 NeuronCores.

Sharding: core c handles batch b = c//2 and query rows [ (c%2)*S/2, (c%2+1)*S/2 ).
Each core computes the full KV-side projections for its batch (duplicated between
the two cores sharing a batch) and the Q-side projections / attention / output
projection for its query half. No collectives; the host gathers the 8 output
shards.

Layout strategy: activations are kept feature-major ("transposed", [feature, seq])
so every matmul's contraction dim lands on SBUF partitions. Attention output is
produced directly as attT[h*128+d, q] (v as stationary operand, expT as moving),
which is exactly the lhsT layout the output projection needs - no PE transposes
anywhere. Softmax skips the max-subtraction (scores here are O(1); exp is safe)
and the denominator comes from an all-ones stationary matmul over expT.

RoPE is folded into companion weight matrices host-side:
  rope(x)[2i]   = x[2i] cos_i - x[2i+1] sin_i
  rope(x)[2i+1] = x[2i+1] cos_i + x[2i] sin_i
so with xr = x @ Wr where Wr[:,2i] = -W[:,2i+1], Wr[:,2i+1] = W[:,2i]:
  rope(x @ W) = (x @ W) * cosP + (x @ Wr) * sinP   (pure elementwise).

All matmuls run in bf16 (fp32 PSUM accumulation): same 1 cycle/row streaming
rate as float32r at N>=256, but LDWEIGHTS gets fast-weight-load (disabled for
fp32 dtypes) so the per-matmul weight swap hides under the previous matmul,
and DMA bytes / SBUF footprint halve. qT and the zero-padded per-head qrT2
stay resident in SBUF; q_rot uses a full 128-row krT stationary against
qrT2's zero pad rows (64-row stationaries measured +110ns/matmul).

Scheduling notes (each measured on HW):
- A DMA occupies its issuing engine's queue for the whole transfer, so
  traffic is spread: P1 weights + tables on the ACT ring, activation
  streams / attT / outputs on the SP ring, attention-phase weight
  prefetches and SBUF shuffles on the gpsimd SWDGE ring.
- kT/v for head h+1 are produced during head h so their PSUM evictions
  (ACT engine) sit ahead of the slow [128,512] reciprocals in engine FIFOs.
- psS has 3 banks so the scheduler can hoist the next kc's score matmul
  over the ~800ns exp latency; phase-1/3 chains share its tag, the
  produce_v/k chains take the single psA bank, att@v and the softmax
  denominator accumulate in psG/psZ (2+2).
"""

import sys
import numpy as np

sys.path.insert(0, "/opt/trn_rl_repo")

from contextlib import ExitStack  # noqa: E402

import concourse.bass as bass  # noqa: E402
import concourse.mybir as mybir  # noqa: E402
import concourse.tile as tile  # noqa: E402

F32 = mybir.dt.float32
BF = mybir.dt.bfloat16
AF = mybir.ActivationFunctionType
ALU = mybir.AluOpType

# Max sync-waits walrus CoreV3 codegen accepts on one instruction. The stock
# TileContext tail-drain attaches one wait per outstanding semaphore to a
# single Drain, which this walrus build rejects ("Too many sync wait
# commands"); split across several drains instead.
_MAX_WAITS_PER_INST = 1


def _split_excess_waits_json(bir_json):
    """Walrus CoreV3 codegen rejects instructions carrying more than one
    sync-wait. Tile freely attaches several. Rewrite the BIR: keep one wait on
    the instruction, move the rest onto NoOps inserted just before it on the
    same engine (a same-engine wait that fires earlier is strictly safe).
    Updates are left untouched - they must fire at instruction completion."""
    import orjson

    bir = orjson.loads(bir_json)
    n = 0
    for fn in bir.get("functions", []):
        for bb in fn.get("blocks", []):
            out = []
            for inst in bb.get("instructions", []):
                si = inst.get("sync_info")
                waits = (si or {}).get("on_wait") or []
                if len(waits) > _MAX_WAITS_PER_INST:
                    keep = waits[-_MAX_WAITS_PER_INST:]
                    for w in waits[:-_MAX_WAITS_PER_INST]:
                        out.append({
                            "name": f"I-WS{n}",
                            "opcode": "NoOp",
                            "engine": inst["engine"],
                            "ins": [],
                            "outs": [],
                            "sync_info": {"on_update": [], "on_wait": [w]},
                        })
                        n += 1
                    si["on_wait"] = keep
                out.append(inst)
            bb["instructions"] = out
    return orjson.dumps(bir)


_COMPILE_HOOKED = False


def _install_wait_split_hook():
    """Wrap compile_bir_kernel (both the bass_utils global and the name
    bass2jax imported) so every BIR headed to walrus gets the wait split."""
    global _COMPILE_HOOKED
    if _COMPILE_HOOKED:
        return
    from concourse import bass2jax, bass_utils

    orig = bass_utils.compile_bir_kernel

    def hooked(bir_json, tmpdir, neff_name="file.neff"):
        return orig(_split_excess_waits_json(bir_json), tmpdir, neff_name=neff_name)

    bass_utils.compile_bir_kernel = hooked
    bass2jax.compile_bir_kernel = hooked
    _COMPILE_HOOKED = True


class SplitDrainTileContext(tile.TileContext):
    def _drain_and_barrier(self, tick_clock, wait_clock):
        from concourse.tile_scheduler import N_PROCS
        from concourse.vector_clock import ScopedClock, VectorClock

        g = tick_clock.global_clock
        vals = [g[p] for p in range(N_PROCS)]
        nz = [p for p in range(N_PROCS) if vals[p] > 0]
        groups = [nz[i:i + _MAX_WAITS_PER_INST]
                  for i in range(0, len(nz), _MAX_WAITS_PER_INST)] or [[]]
        for grp in groups:
            sub = VectorClock([vals[p] if p in grp else 0 for p in range(N_PROCS)])
            drain_inst = self.nc.sync.drain()
            wait_clock.add_sem_waits(drain_inst.ins, ScopedClock({None: sub}))

        self.nc.all_engine_barrier()
        assert self.sems is not None
        popped = self.nc._tile_sem_poison_stack.pop()
        assert popped is self._sem_poison
        self.nc.clear_and_free_semaphores(list(self.sems.allocated().values()))
        self.nc.all_engine_barrier()


# ----------------------------------------------------------------------------
# Config
# ----------------------------------------------------------------------------

class Cfg:
    def __init__(self, E=2048, DM=2048, H=16, DC=512, DC1=1536, S=2048, Q=1024,
                 QT=512, bf16=True):
        self.E, self.DM, self.H, self.DC, self.DC1 = E, DM, H, DC, DC1
        self.S, self.Q, self.QT = S, Q, QT
        self.DR = 64          # rotary dim (fixed by the problem)
        self.DH = 128         # nope head dim (fixed: DM // H)
        self.bf16 = bf16
        assert DM == H * self.DH and H % 2 == 0
        assert E % 128 == 0 and DC % 128 == 0 and DC1 % 128 == 0
        assert S % 128 == 0
        assert Q % QT == 0 and Q % 128 == 0 and QT <= 512
        self.EC = E // 128        # embed chunks
        self.CC = DC // 128       # c_kv chunks
        self.C1C = DC1 // 128     # c_q chunks
        self.KC = S // 128        # key chunks (128-wide)
        self.ST = min(512, S)     # seq tile for phase 1
        self.STN = S // self.ST
        self.NT = min(512, S)     # kT free tile
        self.NTN = S // self.NT
        self.QTN = Q // QT
        self.MT = min(512, DM)    # out-proj free tile
        self.MTN = DM // self.MT
        self.QON = Q // 128       # out-proj q tiles


FULL = Cfg()


# ----------------------------------------------------------------------------
# Program builder (single-core SPMD program)
# ----------------------------------------------------------------------------

def build_program(cfg: Cfg, has_buv=True, has_bo=True):
    c = cfg
    FR = BF if getattr(cfg, "bf16", False) else mybir.dt.float32r
    nc = bass.Bass()
    r = lambda ap: ap  # noqa: E731

    # -- DRAM parameters -----------------------------------------------------
    xt = nc.dram_tensor("xt", [c.E, c.S], FR, kind="ExternalInput")
    xtq = nc.dram_tensor("xtq", [c.E, c.Q], FR, kind="ExternalInput")
    cosq = nc.dram_tensor("cosq", [128, c.Q], F32, kind="ExternalInput")
    sinq = nc.dram_tensor("sinq", [128, c.Q], F32, kind="ExternalInput")
    # rows 0:64 cos table, rows 64:128 sin table (packed for the fused k-rope)
    cossink = nc.dram_tensor("cossink", [128, c.S], F32, kind="ExternalInput")
    wdq = nc.dram_tensor("wdq", [c.E, c.DC1], FR, kind="ExternalInput")
    bdq = nc.dram_tensor("bdq", [c.DC1], F32, kind="ExternalInput")
    wdkv = nc.dram_tensor("wdkv", [c.E, c.DC], FR, kind="ExternalInput")
    bdkv = nc.dram_tensor("bdkv", [c.DC], F32, kind="ExternalInput")
    wuq = nc.dram_tensor("wuq", [c.DC1, c.DM], FR, kind="ExternalInput")
    buq = nc.dram_tensor("buq", [c.DM], F32, kind="ExternalInput")
    wrq = nc.dram_tensor("wrq", [c.DC1, c.H * c.DR], FR, kind="ExternalInput")
    brq = nc.dram_tensor("brq", [c.H * c.DR], F32, kind="ExternalInput")
    wrqr = nc.dram_tensor("wrqr", [c.DC1, c.H * c.DR], FR, kind="ExternalInput")
    brqr = nc.dram_tensor("brqr", [c.H * c.DR], F32, kind="ExternalInput")
    wrk = nc.dram_tensor("wrk", [c.E, 2 * c.DR], FR, kind="ExternalInput")
    brk = nc.dram_tensor("brk", [2 * c.DR], F32, kind="ExternalInput")
    wuk = nc.dram_tensor("wuk", [c.DC, c.DM], FR, kind="ExternalInput")
    buk = nc.dram_tensor("buk", [c.DM], F32, kind="ExternalInput")
    wuv = nc.dram_tensor("wuv", [c.DC, c.DM], FR, kind="ExternalInput")
    buv = nc.dram_tensor("buv", [c.DM], FR, kind="ExternalInput")
    wo = nc.dram_tensor("wo", [c.DM, c.DM], FR, kind="ExternalInput")
    bo = nc.dram_tensor("bo", [c.DM], FR, kind="ExternalInput")
    ones_d = nc.dram_tensor("ones_in", [128, 128], FR, kind="ExternalInput")
    out = nc.dram_tensor("out", [c.Q, c.DM], F32, kind="ExternalOutput")
    attd = nc.dram_tensor("attT_scratch", [c.DM, c.Q], BF)

    with SplitDrainTileContext(nc) as tc, ExitStack() as ctx:
        # weights / tables / small SBUF-SBUF shuffles ride the ACT HWDGE
        # ring; activation streams and output writes ride the SP ring; the
        # bulk qrT2 shuffle rides the gpsimd SWDGE ring. Rings drain in
        # parallel, so bulk weight loads never head-block the activation
        # stream (and vice versa).
        wdma = nc.scalar.dma_start
        adma = nc.sync.dma_start
        gdma = nc.gpsimd.dma_start

        # -- persistent pools ------------------------------------------------
        consts = ctx.enter_context(tc.tile_pool(name="consts", bufs=1))
        res = ctx.enter_context(tc.tile_pool(name="res", bufs=1))

        ckvT = res.tile([128, c.CC, c.S], FR, tag="ckvT")     # c_kv^T
        krT = res.tile([128, c.S], BF, tag="krT")             # roped k_rot^T, dup rows
        qT = res.tile([128, c.H, c.Q], FR, tag="qT")          # scaled q^T

        def load_pcol(name, vec, n):
            # [n*128] dram vector -> [128, n] sbuf (per-partition scalars)
            t = consts.tile([128, n], F32, tag=name)
            wdma(out=t, in_=vec.rearrange("(c p) -> p c", p=128))
            return t

        # PSUM pools (8 banks total: 2+2+2+2)
        psA = ctx.enter_context(tc.tile_pool(name="psA", bufs=1, space="PSUM"))
        psS = ctx.enter_context(tc.tile_pool(name="psS", bufs=3, space="PSUM"))
        psG = ctx.enter_context(tc.tile_pool(name="psG", bufs=2, space="PSUM"))
        psZ = ctx.enter_context(tc.tile_pool(name="psZ", bufs=2, space="PSUM"))

        paq = ctx.enter_context(tc.tile_pool(name="paq", bufs=1))
        # per-head zero-padded q_rot: rows 0:64 = head h's roped q_rot, rows
        # 64:128 = 0. Score matmuls can then use the full 128-row krT
        # stationary (the dup rows multiply zeros), which keeps LDWEIGHTS on
        # the fast path - 64-row stationaries measured +110ns per matmul.
        qrT2 = paq.tile([128, c.H, c.Q], BF, tag="qrT2")

        # head 0/1 attention weights live below the phase pools so their
        # DMAs (issued during 1c) never wait on an aliased zone
        hw = ctx.enter_context(tc.tile_pool(name="hw", bufs=3))

        # ==================================================================
        # Phase 1a: c_kv^T and roped k_rot^T over the full sequence
        # ==================================================================
        with tc.tile_pool(name="p1ax", bufs=2 * c.EC + 2) as p1ax, \
             tc.tile_pool(name="p1aw", bufs=c.EC) as p1aw, \
             tc.tile_pool(name="p1am", bufs=1) as p1am, \
             tc.tile_pool(name="p1at", bufs=4) as p1at:

            # ACT-ring issue order = need order: the 16 wdkv tiles gate the
            # first matmul chain, bdkv its PSUM eviction, wrk/cossink the
            # k_rot tail; everything else is needed phases later.
            wdkv_t, wrk_t = [], []
            for e in range(c.EC):
                wt = p1aw.tile([128, c.DC], FR, tag="wdkv", name=f"wdkv{e}")
                wdkv_t.append(wt)
            for e in range(c.EC):
                wdma(out=wdkv_t[e], in_=wdkv[e * 128:(e + 1) * 128, :])
            bdkv_sb = load_pcol("bdkv", bdkv, c.CC)
            for e in range(c.EC):
                rt = p1aw.tile([128, 2 * c.DR], FR, tag="wrk", name=f"wrk{e}")
                wrk_t.append(rt)
            for e in range(c.EC):
                wdma(out=wrk_t[e], in_=wrk[e * 128:(e + 1) * 128, :])
            brk_sb = load_pcol("brk", brk, 1)  # rows 0:64 brk, 64:128 companion
            coss_sb = p1am.tile([128, c.S], F32, tag="coss")
            wdma(out=coss_sb, in_=cossink[:, :])
            bdq_sb = load_pcol("bdq", bdq, c.C1C)
            buq_sb = load_pcol("buq", buq, c.H)
            brq_sb = load_pcol("brq", brq, c.H // 2)
            brqr_sb = load_pcol("brqr", brqr, c.H // 2)
            buk_sb = load_pcol("buk", buk, c.H)
            buv_sb = consts.tile([1, c.DM], FR, tag="buv")
            wdma(out=buv_sb, in_=buv[:].unsqueeze(0))
            bo_sb = consts.tile([1, c.DM], FR, tag="bo")
            wdma(out=bo_sb, in_=bo[:].unsqueeze(0))
            ones128 = consts.tile([128, 128], FR, tag="ones128")
            wdma(out=ones128, in_=ones_d[:, :])
            ones1 = ones128[0:1, :]

            for st in range(c.STN):
                ssl = bass.ts(st, c.ST)
                xts = []
                for e in range(c.EC):
                    t = p1ax.tile([128, c.ST], FR, tag="xt")
                    adma(out=t, in_=xt[e * 128:(e + 1) * 128, ssl])
                    xts.append(t)
                for ct in range(c.CC):
                    ps = psS.tile([128, c.ST], F32, tag="s")
                    for e in range(c.EC):
                        nc.tensor.matmul(ps, r(wdkv_t[e][:, ct * 128:(ct + 1) * 128]),
                                         r(xts[e]), start=(e == 0), stop=(e == c.EC - 1))
                    nc.vector.tensor_scalar_add(ckvT[:, ct, ssl], ps,
                                                bdkv_sb[:, ct:ct + 1])
                # k_rot: one 128-col stationary covers A rows (0:64, cos
                # part) and companion Ar rows (64:128, sin part) in one psum
                ps = psS.tile([128, c.ST], F32, tag="s")
                for e in range(c.EC):
                    nc.tensor.matmul(ps, r(wrk_t[e]), r(xts[e]),
                                     start=(e == 0), stop=(e == c.EC - 1))
                tmp = p1at.tile([128, c.ST], F32, tag="ktmp")
                nc.vector.scalar_tensor_tensor(tmp[0:64, :], ps[0:64, :],
                                               brk_sb[0:64, :],
                                               coss_sb[0:64, ssl], ALU.add, ALU.mult)
                nc.vector.scalar_tensor_tensor(tmp[64:128, :], ps[64:128, :],
                                               brk_sb[64:128, :],
                                               coss_sb[64:128, ssl], ALU.add, ALU.mult)
                tmp2 = p1at.tile([64, c.ST], F32, tag="ktmp2")
                gdma(out=tmp2, in_=tmp[64:128, :])
                nc.vector.tensor_add(krT[0:64, ssl], tmp[0:64, :], tmp2)
            # duplicate kr rows: rows 64:128 are the stationary rows that
            # multiply qrT2's zero rows - any finite value works, a copy is
            # the cheapest way to guarantee finite.
            gdma(out=krT[64:128, :], in_=krT[0:64, :])

        # zero qrT2's pad rows off the k_rot tail's critical path; the rot
        # matmuls (attention) are the only readers
        nc.gpsimd.memset(qrT2[64:128, :, :], 0.0)

        # pre-issue head 0/1 attention weights: the hw zone aliases nothing,
        # so these flow on the ACT ring during 1b/1c
        wuv_p = hw.tile([128, c.CC, 256], FR, tag="wuv", name="wuv_pre")
        wdma(out=wuv_p,
             in_=wuv.rearrange("(cc p) m -> p cc m", p=128)[:, :, 0:256])
        wuk_p0 = hw.tile([128, c.CC, 128], FR, tag="wuk", name="wuk_pre0")
        wdma(out=wuk_p0,
             in_=wuk.rearrange("(cc p) m -> p cc m", p=128)[:, :, 0:128])
        wuk_p1 = hw.tile([128, c.CC, 128], FR, tag="wuk", name="wuk_pre1")
        wdma(out=wuk_p1,
             in_=wuk.rearrange("(cc p) m -> p cc m", p=128)[:, :, 128:256])

        with tc.tile_pool(name="p1bx", bufs=c.QTN * c.EC + 2) as p1bx, \
             tc.tile_pool(name="p1bw", bufs=2) as p1bw:
            # 1b's activations: fresh zone, so these queue dep-free on the
            # SP ring right behind 1a's xt stream
            xqs = {}
            for qt in range(c.QTN):
                qsl = bass.ts(qt, c.QT)
                for e in range(c.EC):
                    t = p1bx.tile([128, c.QT], FR, tag="xq")
                    adma(out=t, in_=xtq[e * 128:(e + 1) * 128, qsl])
                    xqs[qt, e] = t

            # ==============================================================
            # Phase 1b/1c: c_q^T, then q^T (scaled) and roped q_rot^T
            # ==============================================================
            with tc.tile_pool(name="pcq", bufs=1) as pcq, \
                 tc.tile_pool(name="p1cm", bufs=1) as p1cm, \
                 tc.tile_pool(name="p1cw", bufs=2) as p1cw:
                cqT = pcq.tile([128, c.C1C, c.Q], FR, tag="cqT")

                cosq_sb = p1cm.tile([128, c.Q], F32, tag="cosq")
                sinq_sb = p1cm.tile([128, c.Q], F32, tag="sinq")
                wdma(out=cosq_sb, in_=cosq[:, :])
                wdma(out=sinq_sb, in_=sinq[:, :])

                for ct in range(c.C1C):
                    wdq_ct = p1bw.tile([128, c.EC, 128], FR, tag="wdq")
                    wdma(
                        out=wdq_ct,
                        in_=wdq.rearrange("(e p) m -> p e m", p=128)[:, :, ct * 128:(ct + 1) * 128])
                    for qt in range(c.QTN):
                        qsl = bass.ts(qt, c.QT)
                        ps = psS.tile([128, c.QT], F32, tag="s")
                        for e in range(c.EC):
                            nc.tensor.matmul(ps, r(wdq_ct[:, e, :]), r(xqs[qt, e]),
                                             start=(e == 0), stop=(e == c.EC - 1))
                        nc.vector.tensor_scalar_add(cqT[:, ct, qsl], ps,
                                                    bdq_sb[:, ct:ct + 1])

                with tc.tile_pool(name="p1ct", bufs=4) as p1ct:
                    for h in range(c.H):
                        wuq_h = p1cw.tile([128, c.C1C, 128], FR, tag="wuq")
                        wdma(
                            out=wuq_h,
                            in_=wuq.rearrange("(cc p) m -> p cc m", p=128)[:, :, h * 128:(h + 1) * 128])
                        for qt in range(c.QTN):
                            qsl = bass.ts(qt, c.QT)
                            ps = psS.tile([128, c.QT], F32, tag="s")
                            for ct in range(c.C1C):
                                nc.tensor.matmul(ps, r(wuq_h[:, ct, :]), r(cqT[:, ct, qsl]),
                                                 start=(ct == 0), stop=(ct == c.C1C - 1))
                            nc.vector.tensor_scalar_add(qT[:, h, qsl], ps,
                                                        buq_sb[:, h:h + 1])
                    for hp in range(c.H // 2):
                        wrq_hp = p1cw.tile([128, c.C1C, 128], FR, tag="wrq")
                        wdma(
                            out=wrq_hp,
                            in_=wrq.rearrange("(cc p) m -> p cc m", p=128)[:, :, hp * 128:(hp + 1) * 128])
                        wrqr_hp = p1cw.tile([128, c.C1C, 128], FR, tag="wrqr")
                        wdma(
                            out=wrqr_hp,
                            in_=wrqr.rearrange("(cc p) m -> p cc m", p=128)[:, :, hp * 128:(hp + 1) * 128])
                        for qt in range(c.QTN):
                            qsl = bass.ts(qt, c.QT)
                            psa = psS.tile([128, c.QT], F32, tag="s")
                            for ct in range(c.C1C):
                                nc.tensor.matmul(psa, r(wrq_hp[:, ct, :]), r(cqT[:, ct, qsl]),
                                                 start=(ct == 0), stop=(ct == c.C1C - 1))
                            psar = psS.tile([128, c.QT], F32, tag="s")
                            for ct in range(c.C1C):
                                nc.tensor.matmul(psar, r(wrqr_hp[:, ct, :]), r(cqT[:, ct, qsl]),
                                                 start=(ct == 0), stop=(ct == c.C1C - 1))
                            tmp = p1ct.tile([128, c.QT], F32, tag="qtmp")
                            nc.vector.scalar_tensor_tensor(tmp, psa, brq_sb[:, hp:hp + 1],
                                                           cosq_sb[:, qsl], ALU.add, ALU.mult)
                            qrp = p1ct.tile([128, c.QT], BF, tag="qrp")
                            nc.vector.scalar_tensor_tensor(qrp, psar,
                                                           brqr_sb[:, hp:hp + 1],
                                                           sinq_sb[:, qsl], ALU.add, ALU.mult)
                            nc.vector.tensor_add(qrp, qrp, tmp)
                            # pair-packed rows -> per-head zero-padded layout
                            gdma(out=qrT2[0:64, 2 * hp, qsl], in_=qrp[0:64, :])
                            gdma(out=qrT2[0:64, 2 * hp + 1, qsl], in_=qrp[64:128, :])

        # ==================================================================
        # Phase 2: per-head attention, kT/v produced one head ahead so their
        # PSUM evictions sit before the reciprocals in engine FIFOs (v casts
        # go to the scalar engine for the same reason). Phase 3's first four
        # chains run inside head 15 to cover its tail.
        # ==================================================================
        wo_pre = []
        with tc.tile_pool(name="ow", bufs=c.H + 2) as ow, \
             tc.tile_pool(name="oo", bufs=2) as oo, \
             tc.tile_pool(name="oa", bufs=c.QON) as oa:
            aqs = []

            def p3_chain(wo_t, mt, qo):
                msl = bass.ts(mt, c.MT)
                ps = psS.tile([128, c.MT], F32, tag="s")
                for hc in range(c.H):
                    nc.tensor.matmul(ps, r(aqs[qo][:, hc, :]), r(wo_t[hc]),
                                     start=(hc == 0),
                                     stop=(not has_bo and hc == c.H - 1))
                if has_bo:
                    nc.tensor.matmul(ps, r(ones1), r(bo_sb[:, msl]),
                                     start=False, stop=True)
                osb = oo.tile([128, c.MT], F32, tag="osb")
                nc.vector.tensor_copy(osb, ps)
                adma(out=out[qo * 128:(qo + 1) * 128, msl], in_=osb)

            with tc.tile_pool(name="hk", bufs=2) as hk, \
                 tc.tile_pool(name="hv", bufs=4) as hv, \
                 tc.tile_pool(name="he", bufs=4) as he, \
                 tc.tile_pool(name="hr", bufs=2) as hr:

                v_tiles, k_tiles = {}, {}

                def produce_v(hp, pre=None):
                    if pre is None:
                        wuv_hp = hw.tile([128, c.CC, 256], FR, tag="wuv")
                        gdma(
                            out=wuv_hp,
                            in_=wuv.rearrange("(cc p) m -> p cc m", p=128)[:, :, hp * 256:(hp + 1) * 256])
                    else:
                        wuv_hp = pre
                    v0 = hv.tile([128, c.KC, 128], FR, tag="vh")
                    v1 = hv.tile([128, c.KC, 128], FR, tag="vh")
                    for st in range(c.KC):
                        psf = psA.tile([128, c.NT], F32, tag="ps")
                        ps = psf[:, 0:256]
                        for cc in range(c.CC):
                            nc.tensor.matmul(ps, r(ckvT[:, cc, st * 128:(st + 1) * 128]),
                                             r(wuv_hp[:, cc, :]),
                                             start=(cc == 0),
                                             stop=(not has_buv and cc == c.CC - 1))
                        if has_buv:
                            nc.tensor.matmul(ps, r(ones1),
                                             r(buv_sb[:, hp * 256:(hp + 1) * 256]),
                                             start=False, stop=True)
                        nc.scalar.copy(v0[:, st, :], ps[:, 0:128])
                        nc.scalar.copy(v1[:, st, :], ps[:, 128:256])
                    v_tiles[2 * hp], v_tiles[2 * hp + 1] = v0, v1

                def produce_k(h, pre=None):
                    if pre is None:
                        wuk_h = hw.tile([128, c.CC, 128], FR, tag="wuk")
                        gdma(
                            out=wuk_h,
                            in_=wuk.rearrange("(cc p) m -> p cc m", p=128)[:, :, h * 128:(h + 1) * 128])
                    else:
                        wuk_h = pre
                    kT = hk.tile([128, c.S], FR, tag="kT")
                    for nt in range(c.NTN):
                        nsl = bass.ts(nt, c.NT)
                        ps = psA.tile([128, c.NT], F32, tag="ps")
                        for cc in range(c.CC):
                            nc.tensor.matmul(ps, r(wuk_h[:, cc, :]), r(ckvT[:, cc, nsl]),
                                             start=(cc == 0), stop=(cc == c.CC - 1))
                        nc.scalar.add(kT[:, nsl], ps, buk_sb[:, h:h + 1])
                    k_tiles[h] = kT

                produce_v(0, pre=wuv_p)
                produce_k(0, pre=wuk_p0)
                for h in range(c.H):
                    if h % 2 == 0 and h + 2 < c.H:
                        produce_v(h // 2 + 1)
                    if h + 1 < c.H:
                        produce_k(h + 1, pre=wuk_p1 if h == 0 else None)
                    # one wo prefetch per head for phase 3's mt=0
                    t = ow.tile([128, c.MT], FR, tag="wo")
                    gdma(out=t, in_=wo[h * 128:(h + 1) * 128, 0:c.MT])
                    wo_pre.append(t)

                    kT = k_tiles.pop(h)
                    vh = v_tiles.pop(h)
                    for qt in range(c.QTN):
                        qsl = bass.ts(qt, c.QT)
                        gps = psG.tile([128, c.QT], F32, tag="g")
                        zps = psZ.tile([128, c.QT], F32, tag="z")
                        ets, ess = [], []
                        for kc in range(c.KC):
                            ksl = bass.ts(kc, 128)
                            sps = psS.tile([128, c.QT], F32, tag="s")
                            nc.tensor.matmul(sps, r(kT[:, ksl]), r(qT[:, h, qsl]),
                                             start=True, stop=False)
                            nc.tensor.matmul(sps, r(krT[:, ksl]),
                                             r(qrT2[:, h, qsl]),
                                             start=False, stop=True)
                            et = he.tile([128, c.QT], FR, tag="e")
                            nc.scalar.activation(et, sps, AF.Exp)
                            nc.tensor.matmul(gps, r(vh[:, kc, :]), r(et),
                                             start=(kc == 0), stop=(kc == c.KC - 1))
                            ets.append(et)
                            if kc % 2 == 1:
                                # denominator at half the PE cost: DVE pair-sums
                                # feed one ones-matmul per TWO key chunks. Each
                                # matmul is emitted two pairs after its sum, so
                                # the PE FIFO never waits on a fresh pair-sum
                                # and only 3 es buffers are ever live.
                                es = he.tile([128, c.QT], FR, tag="es")
                                nc.gpsimd.tensor_add(es, ets[kc - 1], ets[kc])
                                ess.append(es)
                                if len(ess) >= 3:
                                    i = len(ess) - 3
                                    nc.tensor.matmul(zps, r(ones128), r(ess[i]),
                                                     start=(i == 0), stop=False)
                        for i in (len(ess) - 2, len(ess) - 1):
                            nc.tensor.matmul(zps, r(ones128), r(ess[i]),
                                             start=(i == 0), stop=(i == len(ess) - 1))
                        recip = hr.tile([128, c.QT], F32, tag="recip")
                        nc.vector.reciprocal(recip, zps)
                        asb = hr.tile([128, c.QT], BF, tag="attsb")
                        nc.vector.tensor_mul(asb, gps, recip)
                        adma(out=attd[h * 128:(h + 1) * 128, qsl], in_=asb)
                        if h == c.H - 1 and qt == 0:
                            # every head's qt0 slab is in attd: prefetch
                            # phase 3's first stationaries under qt1's work
                            for qo in range(c.QON // 2):
                                aq = oa.tile([128, c.H, 128], BF, tag="attq")
                                adma(
                                    out=aq,
                                    in_=attd.rearrange("(hc p) q -> p hc q", p=128)[:, :, qo * 128:(qo + 1) * 128])
                                aqs.append(aq)

            # ==============================================================
            # Phase 3: output projection  out[q, m] = attT.T @ wo + bo
            # ==============================================================
            for qo in range(c.QON // 2, c.QON):
                aq = oa.tile([128, c.H, 128], BF, tag="attq")
                adma(
                    out=aq,
                    in_=attd.rearrange("(hc p) q -> p hc q", p=128)[:, :, qo * 128:(qo + 1) * 128])
                aqs.append(aq)
            for mt in range(c.MTN):
                if mt == 0:
                    wo_t = wo_pre
                    qos = range(c.QON)
                else:
                    wo_t = []
                    for hc in range(c.H):
                        t = ow.tile([128, c.MT], FR, tag="wo")
                        gdma(out=t, in_=wo[hc * 128:(hc + 1) * 128, bass.ts(mt, c.MT)])
                        wo_t.append(t)
                    qos = range(c.QON)
                for qo in qos:
                    p3_chain(wo_t, mt, qo)

    return nc


# ----------------------------------------------------------------------------
# Host side: input prep, sharding, gather
# ----------------------------------------------------------------------------

def _rope_tables(seq_len, dim, theta=10000.0):
    inv_freq = 1.0 / (theta ** (np.arange(0, dim, 2, dtype=np.float32) / dim))
    t = np.arange(seq_len, dtype=np.float32)
    ang = t[:, None] * inv_freq[None, :]  # [S, dim/2]
    return np.cos(ang).astype(np.float32), np.sin(ang).astype(np.float32)


def _rot_companion_cols(w):
    """wr[..., 2i] = -w[..., 2i+1]; wr[..., 2i+1] = w[..., 2i]."""
    wr = np.empty_like(w)
    wr[..., 0::2] = -w[..., 1::2]
    wr[..., 1::2] = w[..., 0::2]
    return wr


def host_inputs(cfg, sequence, W_dkv, b_dkv, W_dq, b_dq, W_uq, b_uq, W_uk, b_uk,
                W_uv, b_uv, W_rq, b_rq, W_rk, b_rk, W_o, b_o):
    """Build the per-core input maps for the SPMD program."""
    c = cfg
    f = lambda a: np.ascontiguousarray(np.asarray(a, dtype=np.float32))  # noqa: E731
    sequence = f(sequence)
    B = sequence.shape[0]
    scaler = np.float32(1.0 / np.sqrt(c.DH + c.DR))

    cos, sin = _rope_tables(c.S, c.DR)  # [S, 32]
    # rows 2i and 2i+1 both carry table column i
    cosk = np.repeat(cos.T, 2, axis=0)  # [64, S]
    sink = np.repeat(sin.T, 2, axis=0)

    shared = dict(
        wdq=f(W_dq), bdq=f(b_dq),
        wdkv=f(W_dkv), bdkv=f(b_dkv),
        wuq=f(W_uq) * scaler, buq=f(b_uq) * scaler,
        wrq=f(W_rq) * scaler, brq=f(b_rq) * scaler,
        wrqr=_rot_companion_cols(f(W_rq) * scaler),
        brqr=_rot_companion_cols(f(b_rq) * scaler),
        wrk=np.concatenate([f(W_rk), _rot_companion_cols(f(W_rk))], axis=1),
        brk=np.concatenate([f(b_rk), _rot_companion_cols(f(b_rk))], axis=0),
        wuk=f(W_uk), buk=f(b_uk),
        wuv=f(W_uv), buv=f(b_uv),
        wo=f(W_o), bo=f(b_o),
        cossink=np.concatenate([f(cosk), f(sink)], axis=0),
        ones_in=np.ones((128, 128), np.float32),
    )
    shared = {k: np.ascontiguousarray(v) for k, v in shared.items()}
    mm_keys = {"wdq", "wdkv", "wuq", "wrq", "wrqr", "wrk", "wuk", "wuv", "wo",
               "buv", "bo", "ones_in"}
    if getattr(c, "bf16", False):
        import ml_dtypes
        for k in mm_keys:
            shared[k] = shared[k].astype(ml_dtypes.bfloat16)

    n_cores = 2 * B
    in_maps = []
    for core in range(n_cores):
        b, half = core // 2, core % 2
        xtc = np.ascontiguousarray(sequence[b].T)         # [E, S]
        q0 = half * c.Q
        xtqc = np.ascontiguousarray(xtc[:, q0:q0 + c.Q])  # [E, Q]
        cq = np.tile(np.repeat(cos[q0:q0 + c.Q].T, 2, axis=0), (2, 1))  # [128, Q]
        sq = np.tile(np.repeat(sin[q0:q0 + c.Q].T, 2, axis=0), (2, 1))
        m = dict(shared)
        if getattr(c, "bf16", False):
            import ml_dtypes
            xtc = xtc.astype(ml_dtypes.bfloat16)
            xtqc = xtqc.astype(ml_dtypes.bfloat16)
        m.update(xt=xtc, xtq=xtqc,
                 cosq=np.ascontiguousarray(cq), sinq=np.ascontiguousarray(sq))
        in_maps.append(m)
    return in_maps


_PROG_CACHE = {}


def kernel(**inputs) -> np.ndarray:
    from concourse.bass_utils import run_bass_kernel_spmd

    _install_wait_split_hook()

    cfg = FULL
    has_buv = bool(np.any(np.asarray(inputs["b_uv"])))
    has_bo = bool(np.any(np.asarray(inputs["b_o"])))
    key = ("full", has_buv, has_bo)
    if key not in _PROG_CACHE:
        _PROG_CACHE[key] = build_program(cfg, has_buv=has_buv, has_bo=has_bo)
    nc = _PROG_CACHE[key]

    in_maps = host_inputs(cfg, **inputs)
    n = len(in_maps)
    res = run_bass_kernel_spmd(nc, in_maps, list(range(n)))

    B = n // 2
    S = 2 * cfg.Q
    full = np.empty((B, S, cfg.DM), dtype=np.float32)
    for core in range(n):
        b, half = core // 2, core % 2
        full[b, half * cfg.Q:(half + 1) * cfg.Q, :] = res.results[core]["out"]
    return full
